# revision 23
# baseline (speedup 1.0000x reference)
"""TRN2 Bass kernel for nn_BlockLinear: per token t, x_t [32,128] ->
P(P(x_t@w1)@w2) where P(Y) = reshape(Y.T, (32,128)).

v4 strategy (data-parallel over 8 NeuronCores, 4096 tokens/core):
  - All wire traffic in bf16 (halves HBM bytes; rel-err budget 2e-2 allows it).
  - Host pre-transposes x to XT[k, (t,b)] so the contraction dim k is on
    partitions at DMA time -> NO on-device input transpose.  Host also
    absorbs the final P permutation on download -> NO on-device output
    transpose.  Only the mid-stage P runs on-device.
  - Weight columns are permuted (w*p = w*[:, perm], perm[32a+i] = 4i+a) so
    the mid-stage P becomes a 32x32 diagonal-block transpose.
  - The mid-stage transpose runs as a CUSTOM DVE op (TRANSPOSE_CAST_ANT):
    reshape front-end in 32x32 TRANSPOSE mode + pass-through uop, letting
    one DVE pass read PSUM f32 and write transposed SBUF bf16.  (The stock
    InstStreamTranspose requires same src/dst dtype, which would force a
    separate cast-evacuation stage costing ~10us/chunk of engine time.)
  - Per 256-token chunk: DMA in -> 16x matmul(w1p) -> 8x fused DVE
    transpose-cast (PSUM f32 -> SBUF bf16) -> 16x matmul(w2p) -> 8x Act
    cast-evac -> DMA out.  Stage1(c+1)/stage2(c) interleave in pairs keeps
    PE warm and spreads DVE/Act evenly.
"""
import numpy as np
import ml_dtypes
from contextlib import ExitStack

import concourse.bass as bass
from concourse import bacc
import concourse.tile as tile
from concourse import mybir
from concourse.bass_utils import run_bass_kernel_spmd

F32 = mybir.dt.float32
BF16 = mybir.dt.bfloat16

N_CORES = 8
TOK_PER_CORE = 4096
CHUNK_TOK = 256          # tokens per chunk; free dim = 32*256 = 8192
N = 4096                 # elems per token


# --- custom DVE op: 32x32 block transpose + dtype cast --------------------- #

class _FusedTransposeCast:
    """Duck-typed DveOp: reshape front-end TRANSPOSE + Src0*C0 pass-through.
    With s0=1.0 this is a pure 32x32-block transpose; dtype conversion
    happens at the DVE read/write stages (PSUM f32 in -> SBUF bf16 out)."""

    name = "TRANSPOSE_CAST_ANT"
    subdim = False

    def __init__(self):
        from concourse.dve_spec import Spec, Src0, C0
        self.spec = Spec(
            body=Src0 * C0,
            # elementwise reference (CoreSim-only; HW path never calls it --
            # note it does NOT model the transpose)
            reference=lambda in0, s0, s1, imm2: in0 * s0,
        )
        self._cache = {}

    def compile(self, ver):
        if ver in self._cache:
            return self._cache[ver]
        from concourse.dve_spec import lower
        from concourse.dve_uop import DveOpSpec, OpConfig, TransposeMode
        import concourse.dve_ops as dve_ops_mod
        spec = DveOpSpec(
            name=self.name,
            opcode=dve_ops_mod.get_dve_sub_opcode(self.name),
            uops=lower(self.spec, ver=ver),
            op=OpConfig(transpose_mode=TransposeMode.TRANSPOSE),
            rd1_en=False,
        )
        spec.validate(ver)
        self._cache[ver] = spec
        return spec


def _register_tc_op():
    import concourse.dve_ops as dve_ops_mod
    for op in dve_ops_mod.OPS:
        if op.name == _FusedTransposeCast.name:
            return op
    op = _FusedTransposeCast()
    row = max(dve_ops_mod._SUB_OPCODE_FOR_NAME.values()) + 1
    assert row < 0x20, "no free custom-DVE opcode rows"
    dve_ops_mod._SUB_OPCODE_FOR_NAME[op.name] = row
    dve_ops_mod.OPS.append(op)
    dve_ops_mod.CUSTOM_DVE_SPECS[op.name] = op.spec
    return op


_TC_OP = _register_tc_op()


# --------------------------------------------------------------------------- #

def _perm():
    p = np.zeros(128, np.int64)
    for a in range(4):
        for i in range(32):
            p[32 * a + i] = 4 * i + a
    return p


def _f32_to_bf16_u16(a):
    """Round-to-nearest-even f32 -> bf16 bit pattern (uint16)."""
    u = np.ascontiguousarray(a, np.float32).view(np.uint32)
    r = ((u.astype(np.uint64) + 0x7FFF + ((u >> 16) & 1)) >> 16).astype(np.uint16)
    return r


def _u16_to_f32(u):
    return (u.astype(np.uint32) << 16).view(np.float32)


def build_nc(ntok):
    nchunks = ntok // CHUNK_TOK
    FD = CHUNK_TOK * 32  # free-dim elems per chunk
    nc = bacc.Bacc("TRN2", target_bir_lowering=False, debug=False)
    X = nc.dram_tensor("xt", [128, ntok * 32], BF16, kind="ExternalInput").ap()
    W1 = nc.dram_tensor("w1p", [128, 128], BF16, kind="ExternalInput").ap()
    W2 = nc.dram_tensor("w2p", [128, 128], BF16, kind="ExternalInput").ap()
    OUT = nc.dram_tensor("out", [128, ntok * 32], BF16, kind="ExternalOutput").ap()

    with tile.TileContext(nc) as tc, ExitStack() as ctx:
        wpool = ctx.enter_context(tc.tile_pool(name="w", bufs=1))
        xtp = ctx.enter_context(tc.tile_pool(name="xtp", bufs=3))
        m2p = ctx.enter_context(tc.tile_pool(name="m2p", bufs=2))
        obp = ctx.enter_context(tc.tile_pool(name="obp", bufs=3))
        psp = ctx.enter_context(tc.tile_pool(name="psp", bufs=2, space="PSUM"))

        w1_sb = wpool.tile([128, 128], BF16)
        w2_sb = wpool.tile([128, 128], BF16)
        nc.sync.dma_start(w1_sb[:], W1[:])
        nc.sync.dma_start(w2_sb[:], W2[:])

        m_tiles = {}
        ob_tiles = {}
        NQ = FD // 1024

        def mm_pair(ps, w_sb, src, q):
            nc.tensor.matmul(ps[:, 0:512], w_sb[:],
                             src[:, bass.ts(2 * q, 512)], start=True, stop=True)
            nc.tensor.matmul(ps[:, 512:1024], w_sb[:],
                             src[:, bass.ts(2 * q + 1, 512)],
                             start=True, stop=True)

        for c in range(nchunks + 1):
            # pair-level software pipeline: stage1(c) / stage2(c-1) interleave
            if c < nchunks:
                xt = xtp.tile([128, FD], BF16, tag="xt")
                nc.sync.dma_start(xt[:], X[:, c * FD:(c + 1) * FD])
                m_tiles[c] = m2p.tile([128, FD], BF16, tag="m2", name="m2")
            if c >= 1:
                ob_tiles[c - 1] = obp.tile([128, FD], BF16, tag="ob", name="ob")
            for qq in range(NQ // 2):
                for q in (2 * qq, 2 * qq + 1):
                    if c < nchunks:
                        ps = psp.tile([128, 1024], F32, tag="a")
                        mm_pair(ps, w1_sb, xt, q)
                        nc.vector._custom_dve(
                            _TC_OP, out=m_tiles[c][:, bass.ts(q, 1024)],
                            in0=ps[:], s0=1.0)
                for q in (2 * qq, 2 * qq + 1):
                    if c >= 1:
                        ps = psp.tile([128, 1024], F32, tag="b")
                        mm_pair(ps, w2_sb, m_tiles[c - 1], q)
                        nc.scalar.copy(ob_tiles[c - 1][:, bass.ts(q, 1024)],
                                       ps[:])
            if c >= 1:
                nc.sync.dma_start(OUT[:, (c - 1) * FD:c * FD], ob_tiles[c - 1][:])
                m_tiles.pop(c - 1)
                ob_tiles.pop(c - 1)

    if not nc.is_finalized():
        nc.finalize()
    return nc


_NC_CACHE = {}


def _get_nc(ntok):
    if ntok not in _NC_CACHE:
        _NC_CACHE[ntok] = build_nc(ntok)
    return _NC_CACHE[ntok]


def prepare_in_maps(x, w1, w2):
    """Host-side shard + layout transform. Returns (in_maps, ntok)."""
    xf = np.ascontiguousarray(x, dtype=np.float32).reshape(-1, N)
    ntok_total = xf.shape[0]
    assert ntok_total % N_CORES == 0
    ntok = ntok_total // N_CORES

    perm = _perm()
    w1p = _f32_to_bf16_u16(np.ascontiguousarray(w1, np.float32)[:, perm])
    w2p = _f32_to_bf16_u16(np.ascontiguousarray(w2, np.float32)[:, perm])
    w1p = w1p.view(ml_dtypes.bfloat16)
    w2p = w2p.view(ml_dtypes.bfloat16)

    xu = _f32_to_bf16_u16(xf)  # [T, 4096] u16
    in_maps = []
    for i in range(N_CORES):
        xc = xu[i * ntok:(i + 1) * ntok].reshape(ntok, 32, 128)
        # XT[k, 32 t + b] = x[t, 128 b + k]
        xt = np.ascontiguousarray(xc.transpose(2, 0, 1)).reshape(128, ntok * 32)
        in_maps.append({
            "xt": xt.view(ml_dtypes.bfloat16),
            "w1p": w1p, "w2p": w2p,
        })
    return in_maps, ntok


def postprocess(results, ntok, lead):
    """Gather per-core OT [128, ntok*32] bf16 -> full f32 output."""
    ntok_total = ntok * N_CORES
    out = np.empty((ntok_total, N), np.float32)
    for i in range(N_CORES):
        ot = np.asarray(results[i]["out"]).view(np.uint16)
        # out[t, 128 i2 + 32 a2 + b2] = OT[32 a2 + i2, 32 t + b2]
        ot = ot.reshape(4, 32, ntok, 32)              # [a2, i2, t, b2]
        oc = ot.transpose(2, 1, 0, 3).reshape(ntok, N)
        out[i * ntok:(i + 1) * ntok] = _u16_to_f32(np.ascontiguousarray(oc))
    return out.reshape(*lead, N)


def kernel(x, w1, w2):
    """x [8, 4096, 4096] f32; w1, w2 [128, 128] f32 -> [8, 4096, 4096] f32."""
    lead = x.shape[:-1]
    in_maps, ntok = prepare_in_maps(x, w1, w2)
    nc = _get_nc(ntok)
    res = run_bass_kernel_spmd(nc, in_maps, list(range(N_CORES)))
    return postprocess(res.results, ntok, lead)


# revision 24
# speedup vs baseline: 1.0019x; 1.0019x over previous
"""TRN2 Bass kernel for nn_BlockLinear: per token t, x_t [32,128] ->
P(P(x_t@w1)@w2) where P(Y) = reshape(Y.T, (32,128)).

v4 strategy (data-parallel over 8 NeuronCores, 4096 tokens/core):
  - All wire traffic in bf16 (halves HBM bytes; rel-err budget 2e-2 allows it).
  - Host pre-transposes x to XT[k, (t,b)] so the contraction dim k is on
    partitions at DMA time -> NO on-device input transpose.  Host also
    absorbs the final P permutation on download -> NO on-device output
    transpose.  Only the mid-stage P runs on-device.
  - Weight columns are permuted (w*p = w*[:, perm], perm[32a+i] = 4i+a) so
    the mid-stage P becomes a 32x32 diagonal-block transpose.
  - The mid-stage transpose runs as a CUSTOM DVE op (TRANSPOSE_CAST_ANT):
    reshape front-end in 32x32 TRANSPOSE mode + pass-through uop, letting
    one DVE pass read PSUM f32 and write transposed SBUF bf16.  (The stock
    InstStreamTranspose requires same src/dst dtype, which would force a
    separate cast-evacuation stage costing ~10us/chunk of engine time.)
  - Per 256-token chunk: DMA in -> 16x matmul(w1p) -> 8x fused DVE
    transpose-cast (PSUM f32 -> SBUF bf16) -> 16x matmul(w2p) -> 8x Act
    cast-evac -> DMA out.  Stage1(c+1)/stage2(c) interleave in pairs keeps
    PE warm and spreads DVE/Act evenly.
"""
import numpy as np
import ml_dtypes
from contextlib import ExitStack

import concourse.bass as bass
from concourse import bacc
import concourse.tile as tile
from concourse import mybir
from concourse.bass_utils import run_bass_kernel_spmd

F32 = mybir.dt.float32
BF16 = mybir.dt.bfloat16

N_CORES = 8
TOK_PER_CORE = 4096
CHUNK_TOK = 256          # tokens per chunk; free dim = 32*256 = 8192
N = 4096                 # elems per token


# --- custom DVE op: 32x32 block transpose + dtype cast --------------------- #

class _FusedTransposeCast:
    """Duck-typed DveOp: reshape front-end TRANSPOSE + Src0*C0 pass-through.
    With s0=1.0 this is a pure 32x32-block transpose; dtype conversion
    happens at the DVE read/write stages (PSUM f32 in -> SBUF bf16 out)."""

    name = "TRANSPOSE_CAST_ANT"
    subdim = False

    def __init__(self):
        from concourse.dve_spec import Spec, Src0, C0
        self.spec = Spec(
            body=Src0 * C0,
            # elementwise reference (CoreSim-only; HW path never calls it --
            # note it does NOT model the transpose)
            reference=lambda in0, s0, s1, imm2: in0 * s0,
        )
        self._cache = {}

    def compile(self, ver):
        if ver in self._cache:
            return self._cache[ver]
        from concourse.dve_spec import lower
        from concourse.dve_uop import DveOpSpec, OpConfig, TransposeMode
        import concourse.dve_ops as dve_ops_mod
        spec = DveOpSpec(
            name=self.name,
            opcode=dve_ops_mod.get_dve_sub_opcode(self.name),
            uops=lower(self.spec, ver=ver),
            op=OpConfig(transpose_mode=TransposeMode.TRANSPOSE),
            rd1_en=False,
        )
        spec.validate(ver)
        self._cache[ver] = spec
        return spec


def _register_tc_op():
    import concourse.dve_ops as dve_ops_mod
    for op in dve_ops_mod.OPS:
        if op.name == _FusedTransposeCast.name:
            return op
    op = _FusedTransposeCast()
    row = max(dve_ops_mod._SUB_OPCODE_FOR_NAME.values()) + 1
    assert row < 0x20, "no free custom-DVE opcode rows"
    dve_ops_mod._SUB_OPCODE_FOR_NAME[op.name] = row
    dve_ops_mod.OPS.append(op)
    dve_ops_mod.CUSTOM_DVE_SPECS[op.name] = op.spec
    return op


_TC_OP = _register_tc_op()


# --------------------------------------------------------------------------- #

def _perm():
    p = np.zeros(128, np.int64)
    for a in range(4):
        for i in range(32):
            p[32 * a + i] = 4 * i + a
    return p


def _f32_to_bf16_u16(a):
    """Round-to-nearest-even f32 -> bf16 bit pattern (uint16)."""
    u = np.ascontiguousarray(a, np.float32).view(np.uint32)
    r = ((u.astype(np.uint64) + 0x7FFF + ((u >> 16) & 1)) >> 16).astype(np.uint16)
    return r


def _u16_to_f32(u):
    return (u.astype(np.uint32) << 16).view(np.float32)


def build_nc(ntok):
    nchunks = ntok // CHUNK_TOK
    FD = CHUNK_TOK * 32  # free-dim elems per chunk
    nc = bacc.Bacc("TRN2", target_bir_lowering=False, debug=False)
    X = nc.dram_tensor("xt", [128, ntok * 32], BF16, kind="ExternalInput").ap()
    W1 = nc.dram_tensor("w1p", [128, 128], BF16, kind="ExternalInput").ap()
    W2 = nc.dram_tensor("w2p", [128, 128], BF16, kind="ExternalInput").ap()
    OUT = nc.dram_tensor("out", [128, ntok * 32], BF16, kind="ExternalOutput").ap()

    with tile.TileContext(nc) as tc, ExitStack() as ctx:
        wpool = ctx.enter_context(tc.tile_pool(name="w", bufs=1))
        xtp = ctx.enter_context(tc.tile_pool(name="xtp", bufs=3))
        m2p = ctx.enter_context(tc.tile_pool(name="m2p", bufs=2))
        obp = ctx.enter_context(tc.tile_pool(name="obp", bufs=3))
        psp = ctx.enter_context(tc.tile_pool(name="psp", bufs=2, space="PSUM"))

        w1_sb = wpool.tile([128, 128], BF16)
        w2_sb = wpool.tile([128, 128], BF16)
        nc.sync.dma_start(w1_sb[:], W1[:])
        nc.sync.dma_start(w2_sb[:], W2[:])

        m_tiles = {}
        ob_tiles = {}
        NQ = FD // 1024

        def mm_pair(ps, w_sb, src, q):
            nc.tensor.matmul(ps[:, 0:512], w_sb[:],
                             src[:, bass.ts(2 * q, 512)], start=True, stop=True)
            nc.tensor.matmul(ps[:, 512:1024], w_sb[:],
                             src[:, bass.ts(2 * q + 1, 512)],
                             start=True, stop=True)

        for c in range(nchunks + 1):
            # pair-level software pipeline: stage1(c) / stage2(c-1) interleave
            if c < nchunks:
                xt = xtp.tile([128, FD], BF16, tag="xt")
                nc.sync.dma_start(xt[:], X[:, c * FD:(c + 1) * FD])
                m_tiles[c] = m2p.tile([128, FD], BF16, tag="m2", name="m2")
            if c >= 1:
                ob_tiles[c - 1] = obp.tile([128, FD], BF16, tag="ob", name="ob")
            for qq in range(NQ // 2):
                for q in (2 * qq, 2 * qq + 1):
                    if c < nchunks:
                        ps = psp.tile([128, 1024], F32, tag="a")
                        mm_pair(ps, w1_sb, xt, q)
                        nc.vector._custom_dve(
                            _TC_OP, out=m_tiles[c][:, bass.ts(q, 1024)],
                            in0=ps[:], s0=1.0)
                for q in (2 * qq, 2 * qq + 1):
                    if c >= 1:
                        ps = psp.tile([128, 1024], F32, tag="b")
                        mm_pair(ps, w2_sb, m_tiles[c - 1], q)
                        nc.scalar.copy(ob_tiles[c - 1][:, bass.ts(q, 1024)],
                                       ps[:])
                # stream each output half to HBM as soon as its evacs land
                if c >= 1 and qq in (NQ // 4 - 1, NQ // 2 - 1):
                    hh = 0 if qq == NQ // 4 - 1 else 1
                    nc.sync.dma_start(
                        OUT[:, (c - 1) * FD + hh * FD // 2:
                            (c - 1) * FD + (hh + 1) * FD // 2],
                        ob_tiles[c - 1][:, bass.ts(hh, FD // 2)])
            if c >= 1:
                m_tiles.pop(c - 1)
                ob_tiles.pop(c - 1)

    if not nc.is_finalized():
        nc.finalize()
    return nc


_NC_CACHE = {}


def _get_nc(ntok):
    if ntok not in _NC_CACHE:
        _NC_CACHE[ntok] = build_nc(ntok)
    return _NC_CACHE[ntok]


def prepare_in_maps(x, w1, w2):
    """Host-side shard + layout transform. Returns (in_maps, ntok)."""
    xf = np.ascontiguousarray(x, dtype=np.float32).reshape(-1, N)
    ntok_total = xf.shape[0]
    assert ntok_total % N_CORES == 0
    ntok = ntok_total // N_CORES

    perm = _perm()
    w1p = _f32_to_bf16_u16(np.ascontiguousarray(w1, np.float32)[:, perm])
    w2p = _f32_to_bf16_u16(np.ascontiguousarray(w2, np.float32)[:, perm])
    w1p = w1p.view(ml_dtypes.bfloat16)
    w2p = w2p.view(ml_dtypes.bfloat16)

    xu = _f32_to_bf16_u16(xf)  # [T, 4096] u16
    in_maps = []
    for i in range(N_CORES):
        xc = xu[i * ntok:(i + 1) * ntok].reshape(ntok, 32, 128)
        # XT[k, 32 t + b] = x[t, 128 b + k]
        xt = np.ascontiguousarray(xc.transpose(2, 0, 1)).reshape(128, ntok * 32)
        in_maps.append({
            "xt": xt.view(ml_dtypes.bfloat16),
            "w1p": w1p, "w2p": w2p,
        })
    return in_maps, ntok


def postprocess(results, ntok, lead):
    """Gather per-core OT [128, ntok*32] bf16 -> full f32 output."""
    ntok_total = ntok * N_CORES
    out = np.empty((ntok_total, N), np.float32)
    for i in range(N_CORES):
        ot = np.asarray(results[i]["out"]).view(np.uint16)
        # out[t, 128 i2 + 32 a2 + b2] = OT[32 a2 + i2, 32 t + b2]
        ot = ot.reshape(4, 32, ntok, 32)              # [a2, i2, t, b2]
        oc = ot.transpose(2, 1, 0, 3).reshape(ntok, N)
        out[i * ntok:(i + 1) * ntok] = _u16_to_f32(np.ascontiguousarray(oc))
    return out.reshape(*lead, N)


def kernel(x, w1, w2):
    """x [8, 4096, 4096] f32; w1, w2 [128, 128] f32 -> [8, 4096, 4096] f32."""
    lead = x.shape[:-1]
    in_maps, ntok = prepare_in_maps(x, w1, w2)
    nc = _get_nc(ntok)
    res = run_bass_kernel_spmd(nc, in_maps, list(range(N_CORES)))
    return postprocess(res.results, ntok, lead)


# revision 25
# speedup vs baseline: 1.0807x; 1.0786x over previous
"""TRN2 Bass kernel for nn_BlockLinear: per token t, x_t [32,128] ->
P(P(x_t@w1)@w2) where P(Y) = reshape(Y.T, (32,128)).

v4 strategy (data-parallel over 8 NeuronCores, 4096 tokens/core):
  - All wire traffic in bf16 (halves HBM bytes; rel-err budget 2e-2 allows it).
  - Host pre-transposes x to XT[k, (t,b)] so the contraction dim k is on
    partitions at DMA time -> NO on-device input transpose.  Host also
    absorbs the final P permutation on download -> NO on-device output
    transpose.  Only the mid-stage P runs on-device.
  - Weight columns are permuted (w*p = w*[:, perm], perm[32a+i] = 4i+a) so
    the mid-stage P becomes a 32x32 diagonal-block transpose.
  - The mid-stage transpose runs as a CUSTOM DVE op (TRANSPOSE_CAST_ANT):
    reshape front-end in 32x32 TRANSPOSE mode + pass-through uop, letting
    one DVE pass read PSUM f32 and write transposed SBUF bf16.  (The stock
    InstStreamTranspose requires same src/dst dtype, which would force a
    separate cast-evacuation stage costing ~10us/chunk of engine time.)
  - Per 256-token chunk: DMA in -> 16x matmul(w1p) -> 8x fused DVE
    transpose-cast (PSUM f32 -> SBUF bf16) -> 16x matmul(w2p) -> 8x Act
    cast-evac -> DMA out.  Stage1(c+1)/stage2(c) interleave in pairs keeps
    PE warm and spreads DVE/Act evenly.
"""
import numpy as np
import ml_dtypes
from contextlib import ExitStack

import concourse.bass as bass
from concourse import bacc
import concourse.tile as tile
from concourse import mybir
from concourse.bass_utils import run_bass_kernel_spmd

F32 = mybir.dt.float32
BF16 = mybir.dt.bfloat16

N_CORES = 8
TOK_PER_CORE = 4096
CHUNK_TOK = 256          # tokens per chunk; free dim = 32*256 = 8192
N = 4096                 # elems per token


# --- custom DVE op: 32x32 block transpose + dtype cast --------------------- #

class _FusedTransposeCast:
    """Duck-typed DveOp: reshape front-end TRANSPOSE + Src0*C0 pass-through.
    With s0=1.0 this is a pure 32x32-block transpose; dtype conversion
    happens at the DVE read/write stages (PSUM f32 in -> SBUF bf16 out)."""

    name = "TRANSPOSE_CAST_ANT"
    subdim = False

    def __init__(self):
        from concourse.dve_spec import Spec, Src0, C0
        self.spec = Spec(
            body=Src0 * C0,
            # elementwise reference (CoreSim-only; HW path never calls it --
            # note it does NOT model the transpose)
            reference=lambda in0, s0, s1, imm2: in0 * s0,
        )
        self._cache = {}

    def compile(self, ver):
        if ver in self._cache:
            return self._cache[ver]
        from concourse.dve_spec import lower
        from concourse.dve_uop import DveOpSpec, OpConfig, TransposeMode
        import concourse.dve_ops as dve_ops_mod
        spec = DveOpSpec(
            name=self.name,
            opcode=dve_ops_mod.get_dve_sub_opcode(self.name),
            uops=lower(self.spec, ver=ver),
            op=OpConfig(transpose_mode=TransposeMode.TRANSPOSE),
            rd1_en=False,
        )
        spec.validate(ver)
        self._cache[ver] = spec
        return spec


def _register_tc_op():
    import concourse.dve_ops as dve_ops_mod
    for op in dve_ops_mod.OPS:
        if op.name == _FusedTransposeCast.name:
            return op
    op = _FusedTransposeCast()
    row = max(dve_ops_mod._SUB_OPCODE_FOR_NAME.values()) + 1
    assert row < 0x20, "no free custom-DVE opcode rows"
    dve_ops_mod._SUB_OPCODE_FOR_NAME[op.name] = row
    dve_ops_mod.OPS.append(op)
    dve_ops_mod.CUSTOM_DVE_SPECS[op.name] = op.spec
    return op


_TC_OP = _register_tc_op()


# --------------------------------------------------------------------------- #

def _perm():
    p = np.zeros(128, np.int64)
    for a in range(4):
        for i in range(32):
            p[32 * a + i] = 4 * i + a
    return p


def _f32_to_bf16_u16(a):
    """Round-to-nearest-even f32 -> bf16 bit pattern (uint16)."""
    u = np.ascontiguousarray(a, np.float32).view(np.uint32)
    r = ((u.astype(np.uint64) + 0x7FFF + ((u >> 16) & 1)) >> 16).astype(np.uint16)
    return r


def _u16_to_f32(u):
    return (u.astype(np.uint32) << 16).view(np.float32)


def build_nc(ntok):
    nchunks = ntok // CHUNK_TOK
    FD = CHUNK_TOK * 32  # free-dim elems per chunk
    nc = bacc.Bacc("TRN2", target_bir_lowering=False, debug=False)
    X = nc.dram_tensor("xt", [128, ntok * 32], BF16, kind="ExternalInput").ap()
    W1 = nc.dram_tensor("w1p", [128, 128], BF16, kind="ExternalInput").ap()
    W2 = nc.dram_tensor("w2p", [128, 128], BF16, kind="ExternalInput").ap()
    OUT = nc.dram_tensor("out", [128, ntok * 32], BF16, kind="ExternalOutput").ap()

    with tile.TileContext(nc) as tc, ExitStack() as ctx:
        wpool = ctx.enter_context(tc.tile_pool(name="w", bufs=1))
        xtp = ctx.enter_context(tc.tile_pool(name="xtp", bufs=3))
        m2p = ctx.enter_context(tc.tile_pool(name="m2p", bufs=2))
        obp = ctx.enter_context(tc.tile_pool(name="obp", bufs=3))
        psp = ctx.enter_context(tc.tile_pool(name="psp", bufs=2, space="PSUM"))

        w1_sb = wpool.tile([128, 128], BF16)
        w2_sb = wpool.tile([128, 128], BF16)
        nc.sync.dma_start(w1_sb[:], W1[:])
        nc.sync.dma_start(w2_sb[:], W2[:])

        m_tiles = {}
        ob_tiles = {}
        NQ = FD // 1024

        def mm_pair(ps, w_sb, src, q):
            nc.tensor.matmul(ps[:, 0:512], w_sb[:],
                             src[:, bass.ts(2 * q, 512)], start=True, stop=True)
            nc.tensor.matmul(ps[:, 512:1024], w_sb[:],
                             src[:, bass.ts(2 * q + 1, 512)],
                             start=True, stop=True)

        for c in range(nchunks + 1):
            # pair-level software pipeline: stage1(c) / stage2(c-1) interleave
            if c < nchunks:
                xt = xtp.tile([128, FD], BF16, tag="xt")
                nc.sync.dma_start(xt[:], X[:, c * FD:(c + 1) * FD])
                m_tiles[c] = m2p.tile([128, FD], BF16, tag="m2", name="m2")
            if c >= 1:
                ob_tiles[c - 1] = obp.tile([128, FD], BF16, tag="ob", name="ob")
            for qq in range(NQ // 2):
                for q in (2 * qq, 2 * qq + 1):
                    if c < nchunks:
                        ps = psp.tile([128, 1024], F32, tag="a")
                        mm_pair(ps, w1_sb, xt, q)
                        nc.vector._custom_dve(
                            _TC_OP, out=m_tiles[c][:, bass.ts(q, 1024)],
                            in0=ps[:], s0=1.0)
                for q in (2 * qq, 2 * qq + 1):
                    if c >= 1:
                        ps = psp.tile([128, 1024], F32, tag="b")
                        mm_pair(ps, w2_sb, m_tiles[c - 1], q)
                        nc.scalar.copy(ob_tiles[c - 1][:, bass.ts(q, 1024)],
                                       ps[:])
                # stream each output half to HBM as soon as its evacs land;
                # issued on the ACT HWDGE ring so input DMAs (SP ring) never
                # queue behind output semaphores
                if c >= 1 and qq in (NQ // 4 - 1, NQ // 2 - 1):
                    hh = 0 if qq == NQ // 4 - 1 else 1
                    nc.scalar.dma_start(
                        OUT[:, (c - 1) * FD + hh * FD // 2:
                            (c - 1) * FD + (hh + 1) * FD // 2],
                        ob_tiles[c - 1][:, bass.ts(hh, FD // 2)])
            if c >= 1:
                m_tiles.pop(c - 1)
                ob_tiles.pop(c - 1)

    if not nc.is_finalized():
        nc.finalize()
    return nc


_NC_CACHE = {}


def _get_nc(ntok):
    if ntok not in _NC_CACHE:
        _NC_CACHE[ntok] = build_nc(ntok)
    return _NC_CACHE[ntok]


def prepare_in_maps(x, w1, w2):
    """Host-side shard + layout transform. Returns (in_maps, ntok)."""
    xf = np.ascontiguousarray(x, dtype=np.float32).reshape(-1, N)
    ntok_total = xf.shape[0]
    assert ntok_total % N_CORES == 0
    ntok = ntok_total // N_CORES

    perm = _perm()
    w1p = _f32_to_bf16_u16(np.ascontiguousarray(w1, np.float32)[:, perm])
    w2p = _f32_to_bf16_u16(np.ascontiguousarray(w2, np.float32)[:, perm])
    w1p = w1p.view(ml_dtypes.bfloat16)
    w2p = w2p.view(ml_dtypes.bfloat16)

    xu = _f32_to_bf16_u16(xf)  # [T, 4096] u16
    in_maps = []
    for i in range(N_CORES):
        xc = xu[i * ntok:(i + 1) * ntok].reshape(ntok, 32, 128)
        # XT[k, 32 t + b] = x[t, 128 b + k]
        xt = np.ascontiguousarray(xc.transpose(2, 0, 1)).reshape(128, ntok * 32)
        in_maps.append({
            "xt": xt.view(ml_dtypes.bfloat16),
            "w1p": w1p, "w2p": w2p,
        })
    return in_maps, ntok


def postprocess(results, ntok, lead):
    """Gather per-core OT [128, ntok*32] bf16 -> full f32 output."""
    ntok_total = ntok * N_CORES
    out = np.empty((ntok_total, N), np.float32)
    for i in range(N_CORES):
        ot = np.asarray(results[i]["out"]).view(np.uint16)
        # out[t, 128 i2 + 32 a2 + b2] = OT[32 a2 + i2, 32 t + b2]
        ot = ot.reshape(4, 32, ntok, 32)              # [a2, i2, t, b2]
        oc = ot.transpose(2, 1, 0, 3).reshape(ntok, N)
        out[i * ntok:(i + 1) * ntok] = _u16_to_f32(np.ascontiguousarray(oc))
    return out.reshape(*lead, N)


def kernel(x, w1, w2):
    """x [8, 4096, 4096] f32; w1, w2 [128, 128] f32 -> [8, 4096, 4096] f32."""
    lead = x.shape[:-1]
    in_maps, ntok = prepare_in_maps(x, w1, w2)
    nc = _get_nc(ntok)
    res = run_bass_kernel_spmd(nc, in_maps, list(range(N_CORES)))
    return postprocess(res.results, ntok, lead)


# revision 26
# speedup vs baseline: 1.0898x; 1.0084x over previous
"""TRN2 Bass kernel for nn_BlockLinear: per token t, x_t [32,128] ->
P(P(x_t@w1)@w2) where P(Y) = reshape(Y.T, (32,128)).

v4 strategy (data-parallel over 8 NeuronCores, 4096 tokens/core):
  - All wire traffic in bf16 (halves HBM bytes; rel-err budget 2e-2 allows it).
  - Host pre-transposes x to XT[k, (t,b)] so the contraction dim k is on
    partitions at DMA time -> NO on-device input transpose.  Host also
    absorbs the final P permutation on download -> NO on-device output
    transpose.  Only the mid-stage P runs on-device.
  - Weight columns are permuted (w*p = w*[:, perm], perm[32a+i] = 4i+a) so
    the mid-stage P becomes a 32x32 diagonal-block transpose.
  - The mid-stage transpose runs as a CUSTOM DVE op (TRANSPOSE_CAST_ANT):
    reshape front-end in 32x32 TRANSPOSE mode + pass-through uop, letting
    one DVE pass read PSUM f32 and write transposed SBUF bf16.  (The stock
    InstStreamTranspose requires same src/dst dtype, which would force a
    separate cast-evacuation stage costing ~10us/chunk of engine time.)
  - Per 256-token chunk: DMA in -> 16x matmul(w1p) -> 8x fused DVE
    transpose-cast (PSUM f32 -> SBUF bf16) -> 16x matmul(w2p) -> 8x Act
    cast-evac -> DMA out.  Stage1(c+1)/stage2(c) interleave in pairs keeps
    PE warm and spreads DVE/Act evenly.
"""
import numpy as np
import ml_dtypes
from contextlib import ExitStack

import concourse.bass as bass
from concourse import bacc
import concourse.tile as tile
from concourse import mybir
from concourse.bass_utils import run_bass_kernel_spmd

F32 = mybir.dt.float32
BF16 = mybir.dt.bfloat16

N_CORES = 8
TOK_PER_CORE = 4096
CHUNK_TOK = 256          # tokens per chunk; free dim = 32*256 = 8192
N = 4096                 # elems per token


# --- custom DVE op: 32x32 block transpose + dtype cast --------------------- #

class _FusedTransposeCast:
    """Duck-typed DveOp: reshape front-end TRANSPOSE + Src0*C0 pass-through.
    With s0=1.0 this is a pure 32x32-block transpose; dtype conversion
    happens at the DVE read/write stages (PSUM f32 in -> SBUF bf16 out)."""

    name = "TRANSPOSE_CAST_ANT"
    subdim = False

    def __init__(self):
        from concourse.dve_spec import Spec, Src0, C0
        self.spec = Spec(
            body=Src0 * C0,
            # elementwise reference (CoreSim-only; HW path never calls it --
            # note it does NOT model the transpose)
            reference=lambda in0, s0, s1, imm2: in0 * s0,
        )
        self._cache = {}

    def compile(self, ver):
        if ver in self._cache:
            return self._cache[ver]
        from concourse.dve_spec import lower
        from concourse.dve_uop import DveOpSpec, OpConfig, TransposeMode
        import concourse.dve_ops as dve_ops_mod
        spec = DveOpSpec(
            name=self.name,
            opcode=dve_ops_mod.get_dve_sub_opcode(self.name),
            uops=lower(self.spec, ver=ver),
            op=OpConfig(transpose_mode=TransposeMode.TRANSPOSE),
            rd1_en=False,
        )
        spec.validate(ver)
        self._cache[ver] = spec
        return spec


def _register_tc_op():
    import concourse.dve_ops as dve_ops_mod
    for op in dve_ops_mod.OPS:
        if op.name == _FusedTransposeCast.name:
            return op
    op = _FusedTransposeCast()
    row = max(dve_ops_mod._SUB_OPCODE_FOR_NAME.values()) + 1
    assert row < 0x20, "no free custom-DVE opcode rows"
    dve_ops_mod._SUB_OPCODE_FOR_NAME[op.name] = row
    dve_ops_mod.OPS.append(op)
    dve_ops_mod.CUSTOM_DVE_SPECS[op.name] = op.spec
    return op


_TC_OP = _register_tc_op()


# --------------------------------------------------------------------------- #

def _perm():
    p = np.zeros(128, np.int64)
    for a in range(4):
        for i in range(32):
            p[32 * a + i] = 4 * i + a
    return p


def _f32_to_bf16_u16(a):
    """Round-to-nearest-even f32 -> bf16 bit pattern (uint16)."""
    u = np.ascontiguousarray(a, np.float32).view(np.uint32)
    r = ((u.astype(np.uint64) + 0x7FFF + ((u >> 16) & 1)) >> 16).astype(np.uint16)
    return r


def _u16_to_f32(u):
    return (u.astype(np.uint32) << 16).view(np.float32)


def build_nc(ntok):
    nchunks = ntok // CHUNK_TOK
    FD = CHUNK_TOK * 32  # free-dim elems per chunk
    nc = bacc.Bacc("TRN2", target_bir_lowering=False, debug=False)
    X = nc.dram_tensor("xt", [128, ntok * 32], BF16, kind="ExternalInput").ap()
    W1 = nc.dram_tensor("w1p", [128, 128], BF16, kind="ExternalInput").ap()
    W2 = nc.dram_tensor("w2p", [128, 128], BF16, kind="ExternalInput").ap()
    OUT = nc.dram_tensor("out", [128, ntok * 32], BF16, kind="ExternalOutput").ap()

    with tile.TileContext(nc) as tc, ExitStack() as ctx:
        wpool = ctx.enter_context(tc.tile_pool(name="w", bufs=1))
        xtp = ctx.enter_context(tc.tile_pool(name="xtp", bufs=3))
        m2p = ctx.enter_context(tc.tile_pool(name="m2p", bufs=3))
        obp = ctx.enter_context(tc.tile_pool(name="obp", bufs=3))
        psp = ctx.enter_context(tc.tile_pool(name="psp", bufs=2, space="PSUM"))

        w1_sb = wpool.tile([128, 128], BF16)
        w2_sb = wpool.tile([128, 128], BF16)
        nc.sync.dma_start(w1_sb[:], W1[:])
        nc.sync.dma_start(w2_sb[:], W2[:])

        m_tiles = {}
        ob_tiles = {}
        NQ = FD // 1024

        def mm_pair(ps, w_sb, src, q):
            nc.tensor.matmul(ps[:, 0:512], w_sb[:],
                             src[:, bass.ts(2 * q, 512)], start=True, stop=True)
            nc.tensor.matmul(ps[:, 512:1024], w_sb[:],
                             src[:, bass.ts(2 * q + 1, 512)],
                             start=True, stop=True)

        for c in range(nchunks + 1):
            # pair-level software pipeline: stage1(c) / stage2(c-1) interleave
            if c < nchunks:
                xt = xtp.tile([128, FD], BF16, tag="xt")
                nc.sync.dma_start(xt[:], X[:, c * FD:(c + 1) * FD])
                m_tiles[c] = m2p.tile([128, FD], BF16, tag="m2", name="m2")
            if c >= 1:
                ob_tiles[c - 1] = obp.tile([128, FD], BF16, tag="ob", name="ob")
            for qq in range(NQ // 2):
                for q in (2 * qq, 2 * qq + 1):
                    if c < nchunks:
                        ps = psp.tile([128, 1024], F32, tag="a")
                        mm_pair(ps, w1_sb, xt, q)
                        nc.vector._custom_dve(
                            _TC_OP, out=m_tiles[c][:, bass.ts(q, 1024)],
                            in0=ps[:], s0=1.0)
                for q in (2 * qq, 2 * qq + 1):
                    if c >= 1:
                        ps = psp.tile([128, 1024], F32, tag="b")
                        mm_pair(ps, w2_sb, m_tiles[c - 1], q)
                        nc.scalar.copy(ob_tiles[c - 1][:, bass.ts(q, 1024)],
                                       ps[:])
                # stream each output half to HBM as soon as its evacs land;
                # issued on the ACT HWDGE ring so input DMAs (SP ring) never
                # queue behind output semaphores
                if c >= 1 and qq in (NQ // 4 - 1, NQ // 2 - 1):
                    hh = 0 if qq == NQ // 4 - 1 else 1
                    nc.scalar.dma_start(
                        OUT[:, (c - 1) * FD + hh * FD // 2:
                            (c - 1) * FD + (hh + 1) * FD // 2],
                        ob_tiles[c - 1][:, bass.ts(hh, FD // 2)])
            if c >= 1:
                m_tiles.pop(c - 1)
                ob_tiles.pop(c - 1)

    if not nc.is_finalized():
        nc.finalize()
    return nc


_NC_CACHE = {}


def _get_nc(ntok):
    if ntok not in _NC_CACHE:
        _NC_CACHE[ntok] = build_nc(ntok)
    return _NC_CACHE[ntok]


def prepare_in_maps(x, w1, w2):
    """Host-side shard + layout transform. Returns (in_maps, ntok)."""
    xf = np.ascontiguousarray(x, dtype=np.float32).reshape(-1, N)
    ntok_total = xf.shape[0]
    assert ntok_total % N_CORES == 0
    ntok = ntok_total // N_CORES

    perm = _perm()
    w1p = _f32_to_bf16_u16(np.ascontiguousarray(w1, np.float32)[:, perm])
    w2p = _f32_to_bf16_u16(np.ascontiguousarray(w2, np.float32)[:, perm])
    w1p = w1p.view(ml_dtypes.bfloat16)
    w2p = w2p.view(ml_dtypes.bfloat16)

    xu = _f32_to_bf16_u16(xf)  # [T, 4096] u16
    in_maps = []
    for i in range(N_CORES):
        xc = xu[i * ntok:(i + 1) * ntok].reshape(ntok, 32, 128)
        # XT[k, 32 t + b] = x[t, 128 b + k]
        xt = np.ascontiguousarray(xc.transpose(2, 0, 1)).reshape(128, ntok * 32)
        in_maps.append({
            "xt": xt.view(ml_dtypes.bfloat16),
            "w1p": w1p, "w2p": w2p,
        })
    return in_maps, ntok


def postprocess(results, ntok, lead):
    """Gather per-core OT [128, ntok*32] bf16 -> full f32 output."""
    ntok_total = ntok * N_CORES
    out = np.empty((ntok_total, N), np.float32)
    for i in range(N_CORES):
        ot = np.asarray(results[i]["out"]).view(np.uint16)
        # out[t, 128 i2 + 32 a2 + b2] = OT[32 a2 + i2, 32 t + b2]
        ot = ot.reshape(4, 32, ntok, 32)              # [a2, i2, t, b2]
        oc = ot.transpose(2, 1, 0, 3).reshape(ntok, N)
        out[i * ntok:(i + 1) * ntok] = _u16_to_f32(np.ascontiguousarray(oc))
    return out.reshape(*lead, N)


def kernel(x, w1, w2):
    """x [8, 4096, 4096] f32; w1, w2 [128, 128] f32 -> [8, 4096, 4096] f32."""
    lead = x.shape[:-1]
    in_maps, ntok = prepare_in_maps(x, w1, w2)
    nc = _get_nc(ntok)
    res = run_bass_kernel_spmd(nc, in_maps, list(range(N_CORES)))
    return postprocess(res.results, ntok, lead)
